# revision 1
# baseline (speedup 1.0000x reference)
"""Trainium2 Bass kernel for the DifferentiableAllocator (Sinkhorn) problem.

Math: the reference runs 200 log-domain Sinkhorn iterations on K = -(C-theta)/eps
with row marginal a and column marginal b.  With E = exp(K - rowmax(K)) and the
identities u = exp(f + rowmax(K)) = a / (E w),  w = exp(g) = b / (E^T u), the
whole loop collapses to scaled matrix-balancing with NO transcendentals:

    w0 = 1;  repeat 200x:  S = E w;  u = a/S;  T = E^T u;  w = b/T
    P = diag(u) E diag(w) / sum(...)

which is exactly what the reference computes (gauge-invariant; validated to
rel err ~5.6e-5 against the fp32 jax reference, 3.5e-6 against fp64).

Folding the constant marginals into the kernel (F = E*b, G = E*a) and writing
the recurrence in terms of rT = 1/T reduces each iteration to 6 DVE ops and
one PE matmul:

    S   = sum_j F[:, j] * rT[j]          (TT mult + grouped free-dim reduce)
    rS  = approx_recip(S)                (1 custom-DVE op; noise is damped
                                          by the Sinkhorn contraction)
    TP  = sum_r G[:, r, :] * rS[:, r]    (TT mult + grouped free-dim reduce)
    TB  = ONES[128x128] @ TP             (one matmul = partition-sum AND
                                          broadcast of T to all partitions)
    rT  = approx_recip(TB)

Layout: rows L=8192 fold onto 128 partitions x 64 rows, 8 columns contiguous
in the free dim.  Runs replicated on all 8 cores (a single serial 200-step
chain over 256 KB of state - sharding would add an allreduce per iteration
for no win); core 0's output is returned.
"""

import numpy as np
from contextlib import ExitStack

import concourse.bass as bass
import concourse.bacc as bacc
import concourse.tile as tile
from concourse import mybir
from concourse.bass_utils import run_bass_kernel_spmd

L, B = 8192, 8
P = 128
R = L // P  # 64 rows per partition
ITERS = 200
EPS_INV = 50.0  # 1/0.02
BITS = (2.0, 3.0, 4.0, 5.0, 6.0, 8.0, 10.0, 12.0)
F32 = mybir.dt.float32
ADD = mybir.AluOpType.add
AXX = mybir.AxisListType.X


def build(iters=ITERS):
    nc = bacc.Bacc("TRN2", target_bir_lowering=False, debug=False)
    theta_d = nc.dram_tensor("theta", [L, B], F32, kind="ExternalInput").ap()
    phi_d = nc.dram_tensor("phi", [B], F32, kind="ExternalInput").ap()
    sens_d = nc.dram_tensor("sens_raw", [L], F32, kind="ExternalInput").ap()
    nraw_d = nc.dram_tensor("n_raw", [L], F32, kind="ExternalInput").ap()
    out_d = nc.dram_tensor("out", [L, B], F32, kind="ExternalOutput").ap()

    with tile.TileContext(nc) as tc, ExitStack() as ctx:
        sb = ctx.enter_context(tc.tile_pool(name="sb", bufs=1))
        ps = ctx.enter_context(tc.tile_pool(name="ps", bufs=2, space="PSUM"))
        ps1 = ctx.enter_context(tc.tile_pool(name="ps1", bufs=1, space="PSUM"))

        # ---- persistent tiles ----
        TH = sb.tile([P, R * B], F32, tag="TH")    # theta
        E = sb.tile([P, R * B], F32, tag="E")      # exp(K - rowmax K)
        F = sb.tile([P, R * B], F32, tag="F")      # E * b
        G = sb.tile([P, R * B], F32, tag="G")      # E * a
        PR = sb.tile([P, R * B], F32, tag="PR")    # row-pass product / scratch
        PR2 = sb.tile([P, R * B], F32, tag="PR2")  # col-pass product
        EB = sb.tile([P, R * B], F32, tag="EB")    # err broadcast
        NR = sb.tile([P, R], F32, tag="NR")        # n_raw -> n
        SR = sb.tile([P, R], F32, tag="SR")        # sens_raw -> n*sens scaled
        A = sb.tile([P, R], F32, tag="A")          # a = n / sum(n)
        S = sb.tile([P, R], F32, tag="S")
        RS = sb.tile([P, R], F32, tag="RS")        # 1/S
        M = sb.tile([P, R], F32, tag="M")          # row min of D
        TP = sb.tile([P, B], F32, tag="TP")        # per-partition col partials
        RTB = sb.tile([P, B], F32, tag="RTB")      # 1/T broadcast on partitions
        BB = sb.tile([P, B], F32, tag="BB")        # b broadcast on partitions
        BT = sb.tile([1, B], F32, tag="BT")        # b (softmax(phi))
        RB = sb.tile([1, B], F32, tag="RB")        # 1/b
        PH = sb.tile([1, B], F32, tag="PH")
        SMALL = sb.tile([1, 4], F32, tag="SMALL")  # misc 1x1 values
        COLP = sb.tile([P, 2], F32, tag="COLP")    # per-partition partials
        SCL = sb.tile([P, 2], F32, tag="SCL")      # per-partition scalars
        ONESC = sb.tile([P, 1], F32, tag="ONESC")  # ones column (matmul lhsT)
        ONESR = sb.tile([1, P], F32, tag="ONESR")  # ones row (matmul lhsT)
        ONES2 = sb.tile([P, P], F32, tag="ONES2")  # ones 128x128 (matmul lhsT)

        def r3(t):  # [P, R*B] -> [P, R, B]
            return t[:].rearrange("p (r j) -> p r j", j=B)

        def c3(t):  # [P, R*B] -> [P, B, R]  (view for per-column reduce)
            return t[:].rearrange("p (r j) -> p j r", j=B)

        def bcast_j(t2):  # [P, R] -> [P, R, B] (step-0 over j)
            return t2[:].unsqueeze(2).broadcast_to((P, R, B))

        def bcast_r(ap2):  # [P, B] AP -> [P, R, B] (step-0 over r)
            return ap2.unsqueeze(1).broadcast_to((P, R, B))

        # ---- loads ----
        nc.sync.dma_start(TH[:], theta_d.rearrange("(p r) j -> p (r j)", p=P))
        nc.sync.dma_start(NR[:], nraw_d.rearrange("(p r) -> p r", p=P))
        nc.sync.dma_start(SR[:], sens_d.rearrange("(p r) -> p r", p=P))
        nc.sync.dma_start(PH[:], phi_d.unsqueeze(0))

        # ---- constants ----
        nc.gpsimd.memset(ONESC[:], 1.0)
        nc.gpsimd.memset(ONESR[:], 1.0)
        nc.gpsimd.memset(ONES2[:], 1.0)
        for j, bits in enumerate(BITS):
            nc.gpsimd.memset(r3(EB)[:, :, j], float(2.0 ** (-2.0 * bits)))

        # ---- setup: n, sens, a, b, E, F, G ----
        # n = n_raw * 1e5 + 1e3
        nc.vector.tensor_scalar(NR[:], NR[:], 1e5, 1e3,
                                op0=mybir.AluOpType.mult, op1=ADD)
        # 1/sum(sens), 1/sum(n): free-reduce, ones-matmul, recip, broadcast
        nc.vector.tensor_reduce(COLP[:, 0:1], SR[:], axis=AXX, op=ADD)
        nc.vector.tensor_reduce(COLP[:, 1:2], NR[:], axis=AXX, op=ADD)
        PSS = ps1.tile([1, 2], F32, tag="small1")
        nc.tensor.matmul(PSS[:], ONESC[:], COLP[:], start=True, stop=True)
        nc.vector.reciprocal(SMALL[:, 0:2], PSS[:])
        PSB = ps1.tile([P, 2], F32, tag="smallP")
        nc.tensor.matmul(PSB[:], ONESR[:], SMALL[:, 0:2], start=True, stop=True)
        nc.vector.tensor_copy(SCL[:], PSB[:])
        # a = n * (1/sum n);  ns = n * sens * (1/sum sens)
        nc.vector.tensor_scalar_mul(A[:], NR[:], SCL[:, 1:2])
        nc.vector.tensor_mul(SR[:], NR[:], SR[:])
        nc.vector.tensor_scalar_mul(SR[:], SR[:], SCL[:, 0:1])
        # D = ns*err - theta (in PR), M = rowmin D, E = exp(50*(M - D))
        nc.vector.tensor_mul(r3(PR), bcast_j(SR), r3(EB))
        nc.vector.tensor_sub(PR[:], PR[:], TH[:])
        nc.vector.tensor_reduce(M[:], r3(PR), axis=AXX, op=mybir.AluOpType.min)
        nc.vector.tensor_sub(r3(E), bcast_j(M), r3(PR))
        nc.scalar.activation(E[:], E[:], mybir.ActivationFunctionType.Exp,
                             scale=EPS_INV)
        # b = softmax(phi)
        nc.vector.tensor_reduce(SMALL[:, 2:3], PH[:], axis=AXX,
                                op=mybir.AluOpType.max)
        nc.scalar.mul(SMALL[:, 3:4], SMALL[:, 2:3], -1.0)
        nc.scalar.activation(BT[:], PH[:], mybir.ActivationFunctionType.Exp,
                             bias=SMALL[:, 3:4], scale=1.0)
        nc.vector.tensor_reduce(SMALL[:, 2:3], BT[:], axis=AXX, op=ADD)
        nc.vector.reciprocal(SMALL[:, 2:3], SMALL[:, 2:3])
        nc.vector.tensor_scalar_mul(BT[:], BT[:], SMALL[:, 2:3])
        nc.vector.reciprocal(RB[:], BT[:])
        # broadcast b and 1/b to all partitions; F = E*b, G = E*a, rT0 = 1/b
        PBB = ps1.tile([P, B], F32, tag="pbb")
        nc.tensor.matmul(PBB[:], ONESR[:], BT[:], start=True, stop=True)
        nc.vector.tensor_copy(BB[:], PBB[:])
        PRB = ps1.tile([P, B], F32, tag="prb")
        nc.tensor.matmul(PRB[:], ONESR[:], RB[:], start=True, stop=True)
        nc.vector.tensor_copy(RTB[:], PRB[:])
        nc.vector.tensor_mul(r3(F), r3(E), bcast_r(BB[:]))
        nc.vector.tensor_mul(r3(G), r3(E), bcast_j(A))

        # ---- sinkhorn iterations ----
        for t in range(iters):
            # S = sum_j F * rT
            nc.vector.tensor_mul(r3(PR), r3(F), bcast_r(RTB[:]))
            nc.vector.tensor_reduce(S[:], r3(PR), axis=AXX, op=ADD)
            nc.vector.reciprocal_approx_fast(RS[:], S[:])
            # T = sum_i G * rS ; TB = ones @ TP sums partitions + broadcasts
            nc.vector.tensor_mul(r3(PR2), r3(G), bcast_j(RS))
            nc.vector.tensor_reduce(TP[:], c3(PR2), axis=AXX, op=ADD)
            tb = ps.tile([P, B], F32, tag="tb")
            nc.tensor.matmul(tb[:], ONES2[:], TP[:], start=True, stop=True)
            nc.vector.reciprocal_approx_fast(RTB[:], tb[:])

        # ---- P = (G*rS) * (b*rT) / total ----
        if iters == 0:  # timing-baseline build only
            nc.gpsimd.memset(PR2[:], 1.0)
        nc.vector.tensor_mul(r3(PR), r3(PR2), bcast_r(RTB[:]))
        nc.vector.tensor_mul(r3(PR), r3(PR), bcast_r(BB[:]))
        nc.vector.tensor_reduce(COLP[:, 0:1], PR[:], axis=AXX, op=ADD)
        PT = ps1.tile([1, 2], F32, tag="small1")
        nc.tensor.matmul(PT[:, 0:1], ONESC[:], COLP[:, 0:1], start=True, stop=True)
        nc.vector.reciprocal(SMALL[:, 0:1], PT[:, 0:1])
        PTB = ps1.tile([P, 2], F32, tag="smallP")
        nc.tensor.matmul(PTB[:, 0:1], ONESR[:], SMALL[:, 0:1], start=True, stop=True)
        nc.vector.tensor_copy(SCL[:, 0:1], PTB[:, 0:1])
        nc.vector.tensor_scalar_mul(PR[:], PR[:], SCL[:, 0:1])
        nc.sync.dma_start(out_d.rearrange("(p r) j -> p (r j)", p=P), PR[:])

    nc.compile()
    return nc


_cache = {}


def _get_nc(iters=ITERS):
    if iters not in _cache:
        _cache[iters] = build(iters)
    return _cache[iters]


def kernel(**inputs):
    nc = _get_nc()
    in_map = {
        "theta": np.ascontiguousarray(inputs["theta"], dtype=np.float32),
        "phi": np.ascontiguousarray(inputs["phi"], dtype=np.float32),
        "sens_raw": np.ascontiguousarray(inputs["sens_raw"], dtype=np.float32),
        "n_raw": np.ascontiguousarray(inputs["n_raw"], dtype=np.float32),
    }
    res = run_bass_kernel_spmd(nc, [dict(in_map) for _ in range(8)],
                               list(range(8)))
    return np.asarray(res.results[0]["out"], dtype=np.float32)



# revision 2
# speedup vs baseline: 1.1881x; 1.1881x over previous
"""Trainium2 Bass kernel for the DifferentiableAllocator (Sinkhorn) problem.

Math: the reference runs 200 log-domain Sinkhorn iterations on K = -(C-theta)/eps
with row marginal a and column marginal b.  With E = exp(K - rowmax(K)) and the
identities u = exp(f + rowmax(K)) = a / (E w),  w = exp(g) = b / (E^T u), the
whole loop collapses to scaled matrix-balancing with NO transcendentals:

    w0 = 1;  repeat 200x:  S = E w;  u = a/S;  T = E^T u;  w = b/T
    P = diag(u) E diag(w) / sum(...)

Folding the marginals in (F = E*b row-major, G = E*a stored j-major) the
iteration is four DVE instructions + one PE matmul:

    segsum: S[r]  = sum_j F[r,j]*rT[j]     (ONE fused mul+grouped-sum op)
    recip:  rS    = approx_recip(S)
    segsum: TP[j] = sum_r G[j,r]*rS[r]     (ONE fused op, j-major layout)
    matmul: TB    = ones[128x128] @ TP     (partition sum + broadcast)
    recip:  rT    = approx_recip(TB)

The fused op is MUL_SEGSUM_ANT — a custom DVE instruction (hand-built uop FSM:
seed -> steady <-> step) computing a per-page running sum of Src0*Src1 that
RESETS at each page boundary; page sums are steered to a contiguous SBUF range
via a scatter-write output AP.  Validated bit-exact vs numpy on HW.

Runs replicated on all 8 cores (a single serial 200-step chain over 256 KB of
state - sharding would add an allreduce per iteration for no win); core 0's
output is returned.  End-to-end rel err vs the fp32 jax reference: 5.4e-5.
"""

import numpy as np
from dataclasses import dataclass
from contextlib import ExitStack

import concourse.bass as bass
import concourse.bacc as bacc
import concourse.tile as tile
from concourse import mybir
from concourse.bass_utils import run_bass_kernel_spmd

# --------------------------------------------------------------------------
# MUL_SEGSUM_ANT: segmented (per-page-reset) running sum of Src0*Src1.
# out[p, s, i] = sum_{i'<=i} in0[p,s,i'] * in1[p,s,i'] for [P, S, N] inputs.
# The public Spec language has no segmented scan; build the 3-state FSM by
# hand exactly the way dve_spec does for PageIdx ops, but with the step state
# re-initialising the scan stage from the boundary element's product.
# --------------------------------------------------------------------------
from concourse import dve_spec as dsp
from concourse import dve_ops
from concourse.dve_spec import (
    Spec, Src0, Src1, AluOp, scan, _collect, _hoist_stream_invariant_ops,
    _validate_body, _build_placement, _scan_overrides, _assemble, _State,
    _Stage, COUNT_ONCE,
)
from concourse.dve_uop import Trigger, N_STAGES, N_LANES, DveVer
from concourse.dve_ops import DveOpSpec, DveOp


def _segsum_lower(spec: Spec, ver: DveVer):
    _validate_body(spec, ver)
    spec2 = _hoist_stream_invariant_ops(spec)
    scans = _collect(spec2.body, dsp.Scan)
    assert len(scans) == 1
    placement = _build_placement(spec2, scans, N_STAGES[ver], N_LANES[ver])
    seed_ov, _ = _scan_overrides(scans, placement.node_stage)
    sc = scans[0]
    d = placement.node_stage[sc]
    step_ov = {d: _Stage(AluOp.BYPASS, sc.expr)}
    consume = (True, True)
    states = [
        _State(placement=placement, overrides=seed_ov, trigger=COUNT_ONCE,
               repeat=1, next=(1, 0, 0), write_out=False),
        _State(placement=placement, consume=consume,
               trigger=(Trigger.SRC_TENSOR_DONE, Trigger.SUB_DIM_DONE,
                        Trigger.NONE),
               next=(0, 2, 0)),
        _State(placement=placement, consume=consume, overrides=step_ov,
               trigger=(Trigger.SRC_TENSOR_DONE, Trigger.SUB_DIM_DONE,
                        Trigger.COUNT),
               next=(0, 2, 1), repeat=1),
    ]
    out = [_assemble(s) for s in states]
    for u in out:
        u.validate(ver)
    return out


@dataclass(frozen=True)
class _SegDveOp(DveOp):
    def compile(self, ver: DveVer) -> DveOpSpec:
        key = (self.name, ver)
        if (r := dve_ops._COMPILE_CACHE.get(key)) is not None:
            return r
        result = DveOpSpec(
            name=self.name,
            opcode=dve_ops.get_dve_sub_opcode(self.name),
            uops=_segsum_lower(self.spec, ver),
            rd1_en=True,
        )
        dve_ops._COMPILE_CACHE[key] = result
        return result


def _ref_segsum(in0, in1, c0, c1, c2):
    a = np.asarray(in0, np.float32)
    b = np.broadcast_to(np.asarray(in1, np.float32), a.shape)
    return np.cumsum(a * b, axis=-1, dtype=np.float32)


def _register_segsum():
    for op in dve_ops.OPS:
        if op.name == "MUL_SEGSUM_ANT":
            return op
    spec = Spec(body=scan(AluOp.ADD, Src0 * Src1), reference=_ref_segsum)
    op = _SegDveOp("MUL_SEGSUM_ANT", spec, True, {})
    dve_ops.OPS.append(op)
    dve_ops.CUSTOM_DVE_SPECS[op.name] = spec
    dve_ops._SUB_OPCODE_FOR_NAME[op.name] = (
        dve_ops._CUSTOM_DVE_ROW_BASE + len(dve_ops.OPS) - 1)
    assert dve_ops._SUB_OPCODE_FOR_NAME[op.name] < 0x20
    return op


MUL_SEGSUM = _register_segsum()


def mul_segsum(nc, out, in0, in1):
    return nc.vector._custom_dve(MUL_SEGSUM, out=out, in0=in0, in1=in1)


# --------------------------------------------------------------------------
# Kernel
# --------------------------------------------------------------------------
L, B = 8192, 8
P = 128
R = L // P  # 64 rows per partition
ITERS = 200
EPS_INV = 50.0  # 1/0.02
BITS = (2.0, 3.0, 4.0, 5.0, 6.0, 8.0, 10.0, 12.0)
F32 = mybir.dt.float32
ADD = mybir.AluOpType.add
AXX = mybir.AxisListType.X


def build(iters=ITERS):
    nc = bacc.Bacc("TRN2", target_bir_lowering=False, debug=False)
    theta_d = nc.dram_tensor("theta", [L, B], F32, kind="ExternalInput").ap()
    phi_d = nc.dram_tensor("phi", [B], F32, kind="ExternalInput").ap()
    sens_d = nc.dram_tensor("sens_raw", [L], F32, kind="ExternalInput").ap()
    nraw_d = nc.dram_tensor("n_raw", [L], F32, kind="ExternalInput").ap()
    out_d = nc.dram_tensor("out", [L, B], F32, kind="ExternalOutput").ap()

    with tile.TileContext(nc) as tc, ExitStack() as ctx:
        sb = ctx.enter_context(tc.tile_pool(name="sb", bufs=1))
        ps = ctx.enter_context(tc.tile_pool(name="ps", bufs=2, space="PSUM"))
        ps1 = ctx.enter_context(tc.tile_pool(name="ps1", bufs=1, space="PSUM"))

        # ---- persistent tiles ----
        TH = sb.tile([P, R * B], F32, tag="TH")    # theta
        E = sb.tile([P, R * B], F32, tag="E")      # exp(K - rowmax K)
        F = sb.tile([P, R * B], F32, tag="F")      # E * b   (r-major)
        GC = sb.tile([P, R * B], F32, tag="GC")    # E * a   (j-major)
        PR = sb.tile([P, R * B], F32, tag="PR")    # row-pass scratch (cumsums)
        PR2 = sb.tile([P, R * B], F32, tag="PR2")  # col-pass scratch
        EB = sb.tile([P, R * B], F32, tag="EB")    # err broadcast
        NR = sb.tile([P, R], F32, tag="NR")        # n_raw -> n
        SR = sb.tile([P, R], F32, tag="SR")        # sens_raw -> n*sens scaled
        A = sb.tile([P, R], F32, tag="A")          # a = n / sum(n)
        RS = sb.tile([P, R], F32, tag="RS")        # 1/S
        M = sb.tile([P, R], F32, tag="M")          # row min of D
        RTB = sb.tile([P, B], F32, tag="RTB")      # 1/T broadcast on partitions
        BB = sb.tile([P, B], F32, tag="BB")        # b broadcast on partitions
        BRT = sb.tile([P, B], F32, tag="BRT")      # b * rT (finale)
        BT = sb.tile([1, B], F32, tag="BT")        # b (softmax(phi))
        RB = sb.tile([1, B], F32, tag="RB")        # 1/b
        PH = sb.tile([1, B], F32, tag="PH")
        SMALL = sb.tile([1, 4], F32, tag="SMALL")  # misc 1x1 values
        COLP = sb.tile([P, 2], F32, tag="COLP")    # per-partition partials
        SCL = sb.tile([P, 2], F32, tag="SCL")      # per-partition scalars
        ONESC = sb.tile([P, 1], F32, tag="ONESC")  # ones column (matmul lhsT)
        ONESR = sb.tile([1, P], F32, tag="ONESR")  # ones row (matmul lhsT)
        ONES2 = sb.tile([P, P], F32, tag="ONES2")  # ones 128x128 (matmul lhsT)

        def r3(t):  # [P, R*B] -> [P, R, B]  (r-major pages of B)
            return t[:].rearrange("p (r j) -> p r j", j=B)

        def j3(t):  # [P, R*B] -> [P, B, R]  (j-major pages of R)
            return t[:].rearrange("p (j r) -> p j r", r=R)

        def bcast_j(t2):  # [P, R] -> [P, R, B] (page-stride-0 over j)
            return t2[:].unsqueeze(2).broadcast_to((P, R, B))

        def bcast_r(ap2):  # [P, B] AP -> [P, R, B] (page-stride-0 over r)
            return ap2.unsqueeze(1).broadcast_to((P, R, B))

        def bcast_rj(t2):  # [P, R] -> [P, B, R] (j-major: bcast over j pages)
            return t2[:].unsqueeze(1).broadcast_to((P, B, R))

        # scatter-write views: stream element (page s, idx i) lands at
        # address s + S*i, so page sums (last i) occupy a contiguous range.
        def scat_row(t):  # [P, 64 pages, 8] -> sums at [448:512]
            return t[:].rearrange("p (i s) -> p s i", i=B)

        def scat_col(t):  # [P, 8 pages, 64] -> sums at [504:512]
            return t[:].rearrange("p (i s) -> p s i", i=R)

        # ---- loads ----
        nc.sync.dma_start(TH[:], theta_d.rearrange("(p r) j -> p (r j)", p=P))
        nc.sync.dma_start(NR[:], nraw_d.rearrange("(p r) -> p r", p=P))
        nc.sync.dma_start(SR[:], sens_d.rearrange("(p r) -> p r", p=P))
        nc.sync.dma_start(PH[:], phi_d.unsqueeze(0))

        # ---- constants ----
        nc.gpsimd.memset(ONESC[:], 1.0)
        nc.gpsimd.memset(ONESR[:], 1.0)
        nc.gpsimd.memset(ONES2[:], 1.0)
        for j, bits in enumerate(BITS):
            nc.gpsimd.memset(r3(EB)[:, :, j], float(2.0 ** (-2.0 * bits)))

        # ---- setup: n, sens, a, b, E, F, G_c ----
        nc.vector.tensor_scalar(NR[:], NR[:], 1e5, 1e3,
                                op0=mybir.AluOpType.mult, op1=ADD)
        nc.vector.tensor_reduce(COLP[:, 0:1], SR[:], axis=AXX, op=ADD)
        nc.vector.tensor_reduce(COLP[:, 1:2], NR[:], axis=AXX, op=ADD)
        PSS = ps1.tile([1, 2], F32, tag="small1")
        nc.tensor.matmul(PSS[:], ONESC[:], COLP[:], start=True, stop=True)
        nc.vector.reciprocal(SMALL[:, 0:2], PSS[:])
        PSB = ps1.tile([P, 2], F32, tag="smallP")
        nc.tensor.matmul(PSB[:], ONESR[:], SMALL[:, 0:2], start=True, stop=True)
        nc.vector.tensor_copy(SCL[:], PSB[:])
        nc.vector.tensor_scalar_mul(A[:], NR[:], SCL[:, 1:2])
        nc.vector.tensor_mul(SR[:], NR[:], SR[:])
        nc.vector.tensor_scalar_mul(SR[:], SR[:], SCL[:, 0:1])
        # D = ns*err - theta (in PR), M = rowmin D, E = exp(50*(M - D))
        nc.vector.tensor_mul(r3(PR), bcast_j(SR), r3(EB))
        nc.vector.tensor_sub(PR[:], PR[:], TH[:])
        nc.vector.tensor_reduce(M[:], r3(PR), axis=AXX, op=mybir.AluOpType.min)
        nc.vector.tensor_sub(r3(E), bcast_j(M), r3(PR))
        nc.scalar.activation(E[:], E[:], mybir.ActivationFunctionType.Exp,
                             scale=EPS_INV)
        # b = softmax(phi)
        nc.vector.tensor_reduce(SMALL[:, 2:3], PH[:], axis=AXX,
                                op=mybir.AluOpType.max)
        nc.scalar.mul(SMALL[:, 3:4], SMALL[:, 2:3], -1.0)
        nc.scalar.activation(BT[:], PH[:], mybir.ActivationFunctionType.Exp,
                             bias=SMALL[:, 3:4], scale=1.0)
        nc.vector.tensor_reduce(SMALL[:, 2:3], BT[:], axis=AXX, op=ADD)
        nc.vector.reciprocal(SMALL[:, 2:3], SMALL[:, 2:3])
        nc.vector.tensor_scalar_mul(BT[:], BT[:], SMALL[:, 2:3])
        nc.vector.reciprocal(RB[:], BT[:])
        # broadcast b and 1/b to all partitions; F = E*b (r-major);
        # G_c = E*a (j-major, one strided read at setup); rT0 = 1/b
        PBB = ps1.tile([P, B], F32, tag="pbb")
        nc.tensor.matmul(PBB[:], ONESR[:], BT[:], start=True, stop=True)
        nc.vector.tensor_copy(BB[:], PBB[:])
        PRB = ps1.tile([P, B], F32, tag="prb")
        nc.tensor.matmul(PRB[:], ONESR[:], RB[:], start=True, stop=True)
        nc.vector.tensor_copy(RTB[:], PRB[:])
        nc.vector.tensor_mul(r3(F), r3(E), bcast_r(BB[:]))
        nc.vector.tensor_mul(
            j3(GC), E[:].rearrange("p (r j) -> p j r", j=B), bcast_rj(A))

        # ---- sinkhorn iterations: 4 DVE ops + 1 matmul each ----
        SROW = scat_row(PR)
        SCOL = scat_col(PR2)
        for t in range(iters):
            # S = sum_j F * rT; page sums land contiguously at PR[:, 448:512]
            mul_segsum(nc, SROW, r3(F), bcast_r(RTB[:]))
            nc.vector.reciprocal_approx_fast(RS[:], PR[:, R * B - R:])
            # TP = sum_r G * rS; page sums land at PR2[:, 504:512]
            mul_segsum(nc, SCOL, j3(GC), bcast_rj(RS))
            tb = ps.tile([P, B], F32, tag="tb")
            nc.tensor.matmul(tb[:], ONES2[:], PR2[:, R * B - B:],
                             start=True, stop=True)
            nc.vector.reciprocal_approx_fast(RTB[:], tb[:])

        # ---- P = (G*rS) * (b*rT) / total, materialised r-major ----
        if iters == 0:  # timing-baseline build only
            nc.gpsimd.memset(RS[:], 1.0)
        # products G*rS written straight into r-major layout (scatter write)
        nc.vector.tensor_mul(
            PR[:].rearrange("p (r j) -> p j r", j=B), j3(GC), bcast_rj(RS))
        nc.vector.tensor_mul(BRT[:], BB[:], RTB[:])
        nc.vector.tensor_mul(r3(PR), r3(PR), bcast_r(BRT[:]))
        nc.vector.tensor_reduce(COLP[:, 0:1], PR[:], axis=AXX, op=ADD)
        PT = ps1.tile([1, 2], F32, tag="small1")
        nc.tensor.matmul(PT[:, 0:1], ONESC[:], COLP[:, 0:1], start=True, stop=True)
        nc.vector.reciprocal(SMALL[:, 0:1], PT[:, 0:1])
        PTB = ps1.tile([P, 2], F32, tag="smallP")
        nc.tensor.matmul(PTB[:, 0:1], ONESR[:], SMALL[:, 0:1], start=True, stop=True)
        nc.vector.tensor_copy(SCL[:, 0:1], PTB[:, 0:1])
        nc.vector.tensor_scalar_mul(PR[:], PR[:], SCL[:, 0:1])
        nc.sync.dma_start(out_d.rearrange("(p r) j -> p (r j)", p=P), PR[:])

    nc.compile()
    return nc


_cache = {}


def _get_nc(iters=ITERS):
    if iters not in _cache:
        _cache[iters] = build(iters)
    return _cache[iters]


def kernel(**inputs):
    nc = _get_nc()
    in_map = {
        "theta": np.ascontiguousarray(inputs["theta"], dtype=np.float32),
        "phi": np.ascontiguousarray(inputs["phi"], dtype=np.float32),
        "sens_raw": np.ascontiguousarray(inputs["sens_raw"], dtype=np.float32),
        "n_raw": np.ascontiguousarray(inputs["n_raw"], dtype=np.float32),
    }
    res = run_bass_kernel_spmd(nc, [dict(in_map) for _ in range(8)],
                               list(range(8)))
    return np.asarray(res.results[0]["out"], dtype=np.float32)
